# revision 9
# baseline (speedup 1.0000x reference)
"""Trainium2 Bass kernel: segmented mean-pool over ragged bags (nn_Aggregator).

samples [131072, 512] f32, bags_num_samples [64] int -> [64, 512] f32 bag means
(bag i owns a contiguous run of rows; counts semantics match
 jnp.repeat(arange(B), counts, total_repeat_length=T)).

Distribution strategy (8 NeuronCores, SPMD single program):
- Host splits every bag's row range into 256-row PAIRS (two 128-row tiles,
  single-bag by construction) plus leftover rows (<256 per bag). Pairs and
  128-row leftover tiles are dealt round-robin across cores (pad with zero
  rows only to equalize core tile counts, <1% traffic).
- Per core, pair region: DVE adds the two tiles of a pair elementwise, then
  one fp32 TensorE matmul per pair with a one-hot indicator [128, 64]
  (built on-chip via is_equal of the pair's segment id against a bag iota)
  accumulates into a PSUM [64, 512] partial. Pre-adding halves the PE
  stream, which matters because fp32 matmul runs at 4 cycles/row.
- Leftover region: per-tile indicator matmuls; rows of mixed bags are
  handled exactly by the per-row indicator.
- Host sums the 8 partials [64, 512] and divides by counts (fp32), matching
  the reference's segment_sum + divide semantics. All arithmetic fp32-exact.

All ragged-boundary information lives in per-core *data* (segment ids), so
one compiled program serves all cores and any bag-size distribution.
"""
import numpy as np

from concourse import mybir
from concourse.bacc import Bacc
from concourse.tile import TileContext
from concourse.bass_utils import run_bass_kernel_spmd

B = 64          # bags
D = 512         # feature dim
T = 131072      # total rows
N_CORES = 8
P = 128         # SBUF partitions
CHUNK = 8       # 128-row tiles per DMA (2 MiB)

_PROGRAM_CACHE: dict = {}


def _segment_ids(bags_num_samples: np.ndarray) -> np.ndarray:
    """jnp.repeat(arange(B), counts, total_repeat_length=T) semantics:
    truncate if the full repeat exceeds T, pad with the last value if short."""
    counts = np.asarray(bags_num_samples, dtype=np.int64)
    reps = np.repeat(np.arange(counts.shape[0], dtype=np.int64), np.maximum(counts, 0))
    if reps.size >= T:
        return reps[:T]
    pad_val = reps[-1] if reps.size else np.int64(0)
    return np.concatenate([reps, np.full(T - reps.size, pad_val, dtype=np.int64)])


def plan_and_pack(samples: np.ndarray, bags_num_samples: np.ndarray):
    """Build per-core inputs. Returns (in_maps, n_pair_tiles, n_rem_tiles)."""
    samples = np.asarray(samples, dtype=np.float32)
    seg = _segment_ids(bags_num_samples)

    # contiguous runs of equal seg id
    bnd = np.flatnonzero(np.diff(seg)) + 1
    starts = np.concatenate([[0], bnd])
    ends = np.concatenate([bnd, [T]])

    pair_slices = []  # (row_start, seg_id); 256 rows of one bag
    rem_slices = []   # (row_start, row_end) leftovers, mixed bags allowed
    for s, e in zip(starts, ends):
        sid = int(seg[s])
        n_pairs = (e - s) // 256
        for j in range(n_pairs):
            pair_slices.append((s + 256 * j, sid))
        if s + 256 * n_pairs < e:
            rem_slices.append((s + 256 * n_pairs, e))

    n_pairs_g = len(pair_slices)
    npair_core = -(-n_pairs_g // N_CORES) if n_pairs_g else 0

    rem_rows = sum(e - s for s, e in rem_slices)
    n_rem_tiles_g = -(-rem_rows // P) if rem_rows else 0
    nrem_core = -(-n_rem_tiles_g // N_CORES) if n_rem_tiles_g else 0

    n_pair_tiles = 2 * npair_core
    nt = n_pair_tiles + nrem_core
    t_core = nt * P

    if rem_rows:
        rem_x = np.concatenate([samples[s:e] for s, e in rem_slices], axis=0)
        rem_s = np.concatenate([seg[s:e] for s, e in rem_slices], axis=0)
        pad = nrem_core * N_CORES * P - rem_rows
        if pad:
            rem_x = np.concatenate([rem_x, np.zeros((pad, D), np.float32)], axis=0)
            rem_s = np.concatenate([rem_s, np.zeros(pad, rem_s.dtype)], axis=0)
    biota = np.broadcast_to(np.arange(B, dtype=np.float32)[None, :], (P, B))

    in_maps = []
    for i in range(N_CORES):
        xp = np.zeros((t_core, D), dtype=np.float32)
        sp = np.zeros(max(nt, 1), dtype=np.float32)   # per-tile seg (pair region)
        sp_rows = np.zeros((max(nrem_core, 1) * P,), dtype=np.float32)
        my_pairs = pair_slices[i::N_CORES]
        for j, (rs, sid) in enumerate(my_pairs):
            xp[j * 256 : (j + 1) * 256] = samples[rs : rs + 256]
            sp[2 * j] = sid
            sp[2 * j + 1] = sid
        if nrem_core:
            lo = i * nrem_core * P
            xp[n_pair_tiles * P :] = rem_x[lo : lo + nrem_core * P]
            sp_rows[: nrem_core * P] = rem_s[lo : lo + nrem_core * P]

        # consts: [pair-tile seg (nt) | leftover per-row seg^T (nrem_core) | iota (B)]
        seg_pair = np.broadcast_to(sp[None, :nt], (P, nt)) if nt else np.zeros((P, 0), np.float32)
        parts = [seg_pair]
        if nrem_core:
            parts.append(sp_rows[: nrem_core * P].reshape(nrem_core, P).T)
        parts.append(biota)
        consts = np.ascontiguousarray(np.concatenate(parts, axis=1), dtype=np.float32)
        in_maps.append({"x": np.ascontiguousarray(xp), "consts": consts})
    return in_maps, n_pair_tiles, nrem_core


def build_program(
    n_pair_tiles: int,
    n_rem_tiles: int,
    loop_repeats: int = 1,
    xbufs: int = 6,
):
    """loop_repeats > 1 wraps the body in a hardware loop repeating identical
    work — used only for slope-based timing experiments."""
    nt = n_pair_tiles + n_rem_tiles
    t_core = nt * P
    n_pairs = n_pair_tiles // 2
    n_mms = n_pairs + n_rem_tiles
    nconst = nt + n_rem_tiles + B

    nc = Bacc()
    x = nc.dram_tensor("x", [t_core, D], mybir.dt.float32, kind="ExternalInput")
    consts = nc.dram_tensor("consts", [P, nconst], mybir.dt.float32, kind="ExternalInput")
    out = nc.dram_tensor("out", [B, D], mybir.dt.float32, kind="ExternalOutput")
    x_view = x.rearrange("(c p) d -> p c d", p=P)

    with TileContext(nc) as tc:
        with (
            tc.tile_pool(name="const", bufs=1) as const_pool,
            tc.tile_pool(name="xin", bufs=xbufs) as x_pool,
            tc.tile_pool(name="ind", bufs=3) as ind_pool,
            tc.tile_pool(name="pair", bufs=3) as pair_pool,
            tc.tile_pool(name="psum", bufs=1, space="PSUM") as psum_pool,
            tc.tile_pool(name="outp", bufs=1) as out_pool,
        ):
            const_tile = const_pool.tile([P, nconst], mybir.dt.float32)
            nc.sync.dma_start(out=const_tile[:], in_=consts[:])
            segp_tile = const_tile[:, :nt]
            segr_tile = (
                const_tile[:, nt : nt + n_rem_tiles] if n_rem_tiles else None
            )
            iota_tile = const_tile[:, nt + n_rem_tiles :]

            def body():
                acc = psum_pool.tile([B, D], mybir.dt.float32)
                mm = 0

                t0 = 0
                while t0 < n_pair_tiles:  # pair region
                    cnt = min(CHUNK, n_pair_tiles - t0)  # even
                    xt = x_pool.tile([P, CHUNK * D], mybir.dt.float32, tag="xt")
                    nc.sync.dma_start(
                        out=xt[:, : cnt * D].rearrange("p (c d) -> p c d", c=cnt),
                        in_=x_view[:, t0 : t0 + cnt, :],
                    )
                    nq = cnt // 2
                    pair = pair_pool.tile([P, (CHUNK // 2) * D], mybir.dt.float32, tag="pr")
                    for q in range(nq):
                        nc.vector.tensor_tensor(
                            out=pair[:, q * D : (q + 1) * D],
                            in0=xt[:, (2 * q) * D : (2 * q + 1) * D],
                            in1=xt[:, (2 * q + 1) * D : (2 * q + 2) * D],
                            op=mybir.AluOpType.add,
                        )
                    ind = ind_pool.tile([P, CHUNK * B], mybir.dt.float32, tag="ind")
                    for q in range(nq):
                        nc.vector.tensor_tensor(
                            out=ind[:, q * B : (q + 1) * B],
                            in0=segp_tile[:, t0 + 2 * q : t0 + 2 * q + 1].to_broadcast([P, B]),
                            in1=iota_tile[:, :],
                            op=mybir.AluOpType.is_equal,
                        )
                    for q in range(nq):
                        nc.tensor.matmul(
                            acc[:],
                            lhsT=ind[:, q * B : (q + 1) * B],
                            rhs=pair[:, q * D : (q + 1) * D],
                            start=(mm == 0),
                            stop=(mm == n_mms - 1),
                        )
                        mm += 1
                    t0 += cnt

                r0 = 0
                while r0 < n_rem_tiles:  # leftover region, per-row indicators
                    cnt = min(CHUNK, n_rem_tiles - r0)
                    xt = x_pool.tile([P, CHUNK * D], mybir.dt.float32, tag="xt")
                    nc.sync.dma_start(
                        out=xt[:, : cnt * D].rearrange("p (c d) -> p c d", c=cnt),
                        in_=x_view[:, n_pair_tiles + r0 : n_pair_tiles + r0 + cnt, :],
                    )
                    ind = ind_pool.tile([P, CHUNK * B], mybir.dt.float32, tag="ind")
                    for c in range(cnt):
                        nc.vector.tensor_tensor(
                            out=ind[:, c * B : (c + 1) * B],
                            in0=segr_tile[:, r0 + c : r0 + c + 1].to_broadcast([P, B]),
                            in1=iota_tile[:, :],
                            op=mybir.AluOpType.is_equal,
                        )
                    for c in range(cnt):
                        nc.tensor.matmul(
                            acc[:],
                            lhsT=ind[:, c * B : (c + 1) * B],
                            rhs=xt[:, c * D : (c + 1) * D],
                            start=(mm == 0),
                            stop=(mm == n_mms - 1),
                        )
                        mm += 1
                    r0 += cnt

                out_sb = out_pool.tile([B, D], mybir.dt.float32)
                nc.vector.tensor_copy(out=out_sb[:], in_=acc[:])
                nc.sync.dma_start(out=out[:], in_=out_sb[:])

            if loop_repeats == 1:
                body()
            else:
                with tc.For_i(0, loop_repeats, 1):
                    body()

    nc.finalize()
    return nc


def _combine(partials, bags_num_samples: np.ndarray) -> np.ndarray:
    sums = np.sum(np.stack(partials), axis=0, dtype=np.float32)
    counts_f = np.asarray(bags_num_samples)[:, None].astype(np.float32)
    with np.errstate(divide="ignore", invalid="ignore"):
        return (sums / counts_f).astype(np.float32)


def kernel(samples: np.ndarray, bags_num_samples: np.ndarray) -> np.ndarray:
    in_maps, npt, nrt = plan_and_pack(samples, bags_num_samples)
    key = (npt, nrt)
    if key not in _PROGRAM_CACHE:
        _PROGRAM_CACHE[key] = build_program(npt, nrt)
    res = run_bass_kernel_spmd(_PROGRAM_CACHE[key], in_maps, list(range(N_CORES)))
    partials = [res.results[i]["out"] for i in range(N_CORES)]
    return _combine(partials, bags_num_samples)
